# revision 1
# baseline (speedup 1.0000x reference)
"""Trainium2 Bass kernel for nn_DRGCNLayer (gnn_message_passing) — v2.

Design vs v1 (see _transcript): DVE was the bottleneck (80% busy) plus 401
activation-table reloads (sigmoid<->exp). Changes:
  - all activations from the exp table (sigmoid via exp + 1/(1+u)): 1 table load
  - q.bk term cancels in the per-dst softmax -> all bk machinery deleted
  - src gathered FEATURE-major via dma_gather(transpose=True) from two
    int16-indexable row ranges (x[0:32768], x[17232:]) -> no pair-select ops,
    no PE transposes + PSUM evacuation for src
  - one-hot(et) built by tensor_scalar is_equal against DMA-replicated et rows
    (4x DVE mode); a_em one-hots per chunk via tensor_scalar (4x mode)
  - k/v/q columns host-permuted to (d,h) order so every per-(edge,head)
    broadcast keeps a packed innermost dim (2x DVE mode)
  - scores per head via bf16 tree-add over d (packed slices)
  - denominator division + bv add moved to host (acc PSUM DMAed out raw)
"""
import os
os.environ.setdefault("JAX_PLATFORMS", "axon,cpu")
import numpy as np
import ml_dtypes

BF = ml_dtypes.bfloat16
F32 = np.float32

N = 50000
E = 800000
H = 128
NR = 64
NH = 8
HD = 16
P = 128
NCORES = 8
NMAXN = 6656          # per-core node slots (13 * 512)
CH = 16               # chunks (of 128 edges) per block
EPB = CH * P          # 2048 edge slots per block
SPB = 4               # supersteps per block
WSS = 512             # superstep width in edges
KA = 1024             # edge slots gathered from table A (src < 32768)
XSPLIT = 32768        # table A rows [0, 32768); table B rows [17232, 50000)
XB0 = N - XSPLIT      # = 17232, base row of table B
PAD_SLOT = 999.0

# consts_bf16 (cb) column map
IOTA128R = 0
WQ = 128
W1S = 256
WKV = 384             # [wk_dh | wv_dh], 256 cols
RELT = 640            # rows 0:64
RELP = 768            # rows 0:64
IDENT = 896
ONES64 = 1024         # row 0, cols 1024:1088; w2 col at 1088
NCB = 1089
# consts_f32 (cf) column map
IOTA64C = 0           # rows 0:64
B1 = 1
BQ025 = 2
NCF = 3


def _build(nblk, b2val, nsup0=NMAXN // WSS, debug=False):
    import concourse.bass as bass
    import concourse.bacc as bacc
    import concourse.mybir as mybir
    import concourse.tile as tile

    f32 = mybir.dt.float32
    bf16 = mybir.dt.bfloat16
    i16 = mybir.dt.int16
    AF = mybir.ActivationFunctionType
    OP = mybir.AluOpType
    nmaxn = nsup0 * WSS

    nc = bacc.Bacc("TRN2", target_bir_lowering=False, debug=False)

    xg = nc.declare_dram_parameter("xg", [N, H], bf16, isOutput=False)
    xt = nc.declare_dram_parameter("xt", [P, nmaxn], bf16, isOutput=False)
    cb = nc.declare_dram_parameter("cb", [P, NCB], bf16, isOutput=False)
    cf = nc.declare_dram_parameter("cf", [P, NCF], f32, isOutput=False)
    idx = nc.declare_dram_parameter("idx", [P, nblk * P], i16, isOutput=False)
    qix = nc.declare_dram_parameter("qix", [P, nblk * P], i16, isOutput=False)
    dlp = nc.declare_dram_parameter("dlp", [P, nblk * CH], f32, isOutput=False)
    dmp = nc.declare_dram_parameter("dmp", [P, nblk * CH], f32, isOutput=False)
    etr = nc.declare_dram_parameter("etr", [nblk, EPB], bf16, isOutput=False)
    qcat = nc.dram_tensor("qcat", [nmaxn, H], bf16)
    outb = nc.declare_dram_parameter("outb", [nblk * P, 136], f32, isOutput=True)
    if debug:
        dbg = {}
        for nm, shp, dt_ in [
            ("dsrc", [P, EPB], bf16), ("dqv", [4, P, CH * H], bf16), ("dqc", [nmaxn, H], bf16),
            ("dohs", [64, WSS], bf16), ("dtmb", [64, WSS], bf16),
            ("dm1", [P, WSS], bf16), ("dh", [P, WSS], bf16),
            ("dkv", [P, 4 * 256], f32), ("dsc", [P, 32], bf16),
            ("ddyn", [P, 4], bf16), ("det", [P, 32], bf16),
            ("dpay", [P, WSS], bf16), ("daem", [P, WSS], bf16),
        ]:
            dbg[nm] = nc.declare_dram_parameter(nm, shp, dt_, isOutput=True)

    with tile.TileContext(nc) as tc:
        with (
            tc.tile_pool(name="cst", bufs=1) as cst,
            tc.tile_pool(name="sb", bufs=2) as sb,
            tc.tile_pool(name="ps", bufs=1, space="PSUM") as ps,
        ):
            cb_t = cst.tile([P, NCB], bf16)
            nc.sync.dma_start(out=cb_t[:], in_=cb[:])
            cf_t = cst.tile([P, NCF], f32)
            nc.sync.dma_start(out=cf_t[:], in_=cf[:])

            iota128r = cb_t[:, IOTA128R:IOTA128R + 128]
            wq_v = cb_t[:, WQ:WQ + 128]
            w1s_v = cb_t[:, W1S:W1S + 128]
            wkv_v = cb_t[:, WKV:WKV + 256]
            relt_v = cb_t[0:64, RELT:RELT + 128]
            relp_v = cb_t[0:64, RELP:RELP + 128]
            ident_v = cb_t[:, IDENT:IDENT + 128]
            ones64_v = cb_t[0:1, ONES64:ONES64 + 64]
            w2_v = cb_t[:, ONES64 + 64:ONES64 + 65]
            iota64c = cf_t[0:64, IOTA64C:IOTA64C + 1]
            b1_v = cf_t[:, B1:B1 + 1]
            bq_v = cf_t[:, BQ025:BQ025 + 1]

            # ---------------- phase 0: qcat = 0.25*(x@Wq+bq), (d,h) cols ----
            for s0 in range(nsup0):
                xts = sb.tile([P, WSS], bf16, tag="p0xt")
                nc.sync.dma_start(out=xts[:], in_=xt[:, s0 * WSS:(s0 + 1) * WSS])
                qps = ps.tile([P, WSS], f32, tag="hps")
                nc.tensor.matmul(qps[:], lhsT=wq_v, rhs=xts[:], start=True, stop=True)
                qs = sb.tile([P, WSS], bf16, tag="p0qs")
                nc.scalar.activation(out=qs[:], in_=qps[:], func=AF.Identity,
                                     scale=0.25, bias=bq_v)
                for j in range(4):
                    qT = ps.tile([P, P], bf16, tag="smt")
                    nc.tensor.transpose(out=qT[:], in_=qs[:, j * P:(j + 1) * P],
                                        identity=ident_v)
                    qrow = sb.tile([P, P], bf16, tag="p0qrow")
                    nc.vector.tensor_copy(out=qrow[:], in_=qT[:])
                    nc.sync.dma_start(
                        out=qcat[s0 * WSS + j * P: s0 * WSS + (j + 1) * P, :],
                        in_=qrow[:])

            tc.strict_bb_all_engine_barrier()

            # ---------------- phase 1: edges ----------------
            etb = dlb = dmb = srcf = qvf = acc = None
            for sup in range(nblk * SPB):
                b, q = divmod(sup, SPB)
                if q == 0:
                    etb = sb.tile([64, EPB], bf16, tag="etb")
                    nc.sync.dma_start(out=etb[:],
                                      in_=etr[b:b + 1, :].to_broadcast([64, EPB]))
                    dlb = sb.tile([P, CH], f32, tag="dlb")
                    nc.sync.dma_start(out=dlb[:], in_=dlp[:, b * CH:(b + 1) * CH])
                    dmb = sb.tile([P, CH], f32, tag="dmb")
                    nc.sync.dma_start(out=dmb[:], in_=dmp[:, b * CH:(b + 1) * CH])
                    ixt = sb.tile([P, P], i16, tag="ixt")
                    nc.sync.dma_start(out=ixt[:], in_=idx[:, b * P:(b + 1) * P])
                    qxt = sb.tile([P, P], i16, tag="qxt")
                    nc.sync.dma_start(out=qxt[:], in_=qix[:, b * P:(b + 1) * P])
                    srcf = sb.tile([P, EPB], bf16, tag="srcf", bufs=3)
                    nc.gpsimd.dma_gather(
                        srcf[:, 0:KA].rearrange("p (c e) -> p c e", c=1),
                        xg[0:XSPLIT, :], ixt[:, 0:KA // 16], KA, KA, H,
                        transpose=True, single_packet=False)
                    nc.gpsimd.dma_gather(
                        srcf[:, KA:EPB].rearrange("p (c e) -> p c e", c=1),
                        xg[XB0:N, :], ixt[:, KA // 16:2 * (KA // 16)],
                        EPB - KA, EPB - KA, H,
                        transpose=True, single_packet=False)
                    qvf = sb.tile([P, CH, H], bf16, tag="qvf", bufs=3)
                    nc.gpsimd.dma_gather(qvf[:], qcat[:], qxt[:], EPB, EPB, H,
                                         single_packet=False)
                    acc = ps.tile([P, 136], f32, tag="acc")
                    u_t = sb.tile([P, CH], bf16, tag="ut")
                    nc.scalar.activation(out=u_t[:], in_=dmb[:], func=AF.Exp)
                    u1 = sb.tile([P, CH], bf16, tag="u1")
                    nc.vector.tensor_scalar_add(u1[:], u_t[:], 1.0)
                    tmc = sb.tile([P, CH], bf16, tag="tmc")
                    with nc.allow_low_precision(reason="tm in bf16 is plenty"):
                        nc.vector.reciprocal(out=tmc[:], in_=u1[:])

                srcs = srcf[:, q * WSS:(q + 1) * WSS]

                smt = ps.tile([P, 260], bf16, tag="smt")
                tmb = ps.tile([64, WSS], bf16, tag="tmb")
                for j in range(4):
                    nc.tensor.transpose(
                        out=tmb[:, j * P:(j + 1) * P],
                        in_=tmc[:, q * 4 + j:q * 4 + j + 1].to_broadcast([P, 64]),
                        identity=ident_v)

                # ohs = one-hot(et) * tm
                oh = sb.tile([64, WSS], bf16, tag="oh")
                nc.vector.tensor_scalar(out=oh[:],
                                        in0=etb[:, q * WSS:(q + 1) * WSS],
                                        scalar1=iota64c, scalar2=None,
                                        op0=OP.is_equal)
                ohs = sb.tile([64, WSS], bf16, tag="ohs")
                nc.vector.tensor_tensor(out=ohs[:], in0=oh[:], in1=tmb[:],
                                        op=OP.mult)

                # rel*tm (feature-major) and m1 = src*rel*tm
                relps = ps.tile([P, WSS], f32, tag="rk")
                nc.tensor.matmul(relps[:], lhsT=relt_v, rhs=ohs[:],
                                 start=True, stop=True)
                rels = sb.tile([P, WSS], bf16, tag="rels")
                nc.scalar.activation(out=rels[:], in_=relps[:], func=AF.Copy)
                m1 = sb.tile([P, WSS], bf16, tag="m1")
                nc.vector.tensor_tensor(out=m1[:], in0=rels[:], in1=srcs,
                                        op=OP.mult)

                # h = relu(W1s^T src + relp^T ohs + b1)
                hps = ps.tile([P, WSS], f32, tag="hps")
                nc.tensor.matmul(hps[:], lhsT=w1s_v, rhs=srcs, start=True, stop=False)
                nc.tensor.matmul(hps[:], lhsT=relp_v, rhs=ohs[:],
                                 start=False, stop=True)
                h_sb = sb.tile([P, WSS], bf16, tag="hsb")
                nc.scalar.activation(out=h_sb[:], in_=hps[:], func=AF.Relu,
                                     bias=b1_v)

                # k,v edge-major per chunk (cols already (d,h) via wkv)
                kk = ps.tile([P, 4, 128], f32, tag="rk")
                vv = ps.tile([P, 4, 128], f32, tag="vv", bufs=2)
                for j in range(4):
                    nc.tensor.matmul(kk[:, j, :],
                                     lhsT=m1[:, j * P:(j + 1) * P],
                                     rhs=wkv_v[:, 0:128], start=True, stop=True)
                    nc.tensor.matmul(vv[:, j, :],
                                     lhsT=m1[:, j * P:(j + 1) * P],
                                     rhs=wkv_v[:, 128:256], start=True, stop=True)

                # dyn_raw = w2^T h  (row), transpose to columns
                dynps = ps.tile([65, 256], f32, tag="dynps")
                h4 = h_sb[:].rearrange("p (c f) -> p c f", c=4)
                nc.tensor.matmul(dynps[0:1, :], lhsT=w2_v, rhs=h4[:, 0:4:2, :],
                                 start=True, stop=True)
                nc.tensor.matmul(dynps[64:65, :], lhsT=w2_v, rhs=h4[:, 1:4:2, :],
                                 start=True, stop=True)
                dynrow = sb.tile([65, 256], bf16, tag="dynrow")
                nc.scalar.activation(out=dynrow[:], in_=dynps[:], func=AF.Copy)
                nc.tensor.transpose(out=smt[:, 128:193],
                                    in_=dynrow[:, 0:128],
                                    identity=ident_v[0:65, 0:65])
                nc.tensor.transpose(out=smt[:, 194:259],
                                    in_=dynrow[:, 128:256],
                                    identity=ident_v[0:65, 0:65])
                ud = sb.tile([P, 4], bf16, tag="ud")
                nc.scalar.activation(out=ud[:, 0:2], in_=smt[:, 128:193:64],
                                     func=AF.Exp, scale=-1.0, bias=float(-b2val))
                nc.scalar.activation(out=ud[:, 2:4], in_=smt[:, 194:259:64],
                                     func=AF.Exp, scale=-1.0, bias=float(-b2val))
                ud1 = sb.tile([P, 4], bf16, tag="ud1")
                nc.vector.tensor_scalar_add(ud1[:], ud[:], 1.0)
                dync = sb.tile([P, 4], bf16, tag="dync")
                with nc.allow_low_precision(reason="dyn in bf16 is plenty"):
                    nc.vector.reciprocal(out=dync[:], in_=ud1[:])

                # scores: qk elementwise then tree-add over d (within head)
                ksb = sb.tile([P, 4, 128], bf16, tag="ksb")
                nc.scalar.activation(out=ksb[:], in_=kk[:], func=AF.Copy)
                qk = sb.tile([P, WSS], bf16, tag="qk")
                nc.vector.tensor_tensor(
                    out=qk[:].rearrange("p (c f) -> p c f", c=4),
                    in0=qvf[:, q * 4:q * 4 + 4, :],
                    in1=ksb[:],
                    op=OP.mult)
                sc = sb.tile([P, 32], bf16, tag="sc")
                with nc.allow_low_precision(reason="bf16 scores fine"):
                    nc.vector.tensor_reduce(
                        out=sc[:].rearrange("p (c h) -> p c h", c=4),
                        in_=qk[:].rearrange("p (c h d) -> p c h d", c=4, h=NH),
                        axis=mybir.AxisListType.X, op=OP.add)

                # e = exp(scores*dyn); ep = e*dyn
                scd = sb.tile([P, 32], bf16, tag="scd")
                nc.vector.tensor_tensor(
                    out=scd[:].rearrange("p (c h) -> p c h", c=4),
                    in0=sc[:].rearrange("p (c h) -> p c h", c=4),
                    in1=dync[:].unsqueeze(-1).to_broadcast([P, 4, NH]),
                    op=OP.mult)
                paye = sb.tile([P, 4, 136], bf16, tag="paye")
                nc.scalar.activation(out=paye[:, :, 0:8], in_=scd[:].rearrange(
                    "p (c h) -> p c h", c=4), func=AF.Exp)
                ep = sb.tile([P, 32], bf16, tag="ep")
                nc.vector.tensor_tensor(
                    out=ep[:].rearrange("p (c h) -> p c h", c=4),
                    in0=paye[:, :, 0:8],
                    in1=dync[:].unsqueeze(-1).to_broadcast([P, 4, NH]),
                    op=OP.mult)
                # payload = v * ep  ((d,h) order keeps innermost packed)
                nc.vector.tensor_tensor(
                    out=paye[:, :, 8:136].rearrange("p c (d h) -> p c d h",
                                                    d=HD),
                    in0=vv[:].rearrange("p c (d h) -> p c d h", d=HD),
                    in1=ep[:].rearrange("p (c h) -> p c h", c=4).unsqueeze(2)
                        .to_broadcast([P, 4, HD, NH]),
                    op=OP.mult)

                # a_em selection matrices, per chunk via tensor_scalar
                a_em = sb.tile([P, WSS], bf16, tag="aem")
                for j in range(4):
                    nc.vector.tensor_scalar(
                        out=a_em[:, j * P:(j + 1) * P], in0=iota128r,
                        scalar1=dlb[:, q * 4 + j:q * 4 + j + 1], scalar2=None,
                        op0=OP.is_equal)

                # accumulate [e | e*dyn*v] per dst row, one group per chunk
                for j in range(4):
                    ch = q * 4 + j
                    nc.tensor.matmul(acc[:], lhsT=a_em[:, j * P:(j + 1) * P],
                                     rhs=paye[:, j, :],
                                     start=(ch == 0), stop=(ch == CH - 1))

                if debug and sup == 0:
                    nc.sync.dma_start(out=dbg["dsrc"][:], in_=srcf[:])
                if debug and q == 0 and b < 4:
                    nc.sync.dma_start(out=dbg["dqv"][b],
                                      in_=qvf[:].rearrange("p c h -> p (c h)"))
                    nc.sync.dma_start(out=dbg["dohs"][:], in_=ohs[:])
                    tmbe = sb.tile([64, WSS], bf16, tag="tmbe")
                    nc.scalar.activation(out=tmbe[:], in_=tmb[:], func=AF.Copy)
                    nc.sync.dma_start(out=dbg["dtmb"][:], in_=tmbe[:])
                    nc.sync.dma_start(out=dbg["dm1"][:], in_=m1[:])
                    nc.sync.dma_start(out=dbg["dh"][:], in_=h_sb[:])
                    kve = sb.tile([P, 4 * 256], f32, tag="kve")
                    nc.vector.tensor_copy(out=kve[:],
                                          in_=kv[:].rearrange("p c f -> p (c f)"))
                    nc.sync.dma_start(out=dbg["dkv"][:], in_=kve[:])
                    nc.sync.dma_start(out=dbg["dsc"][:], in_=sc[:])
                    nc.sync.dma_start(out=dbg["ddyn"][:], in_=dync[:])
                    nc.sync.dma_start(out=dbg["det"][:], in_=e_t[:])
                    nc.sync.dma_start(out=dbg["dpay"][:], in_=pay[:])
                    nc.sync.dma_start(out=dbg["daem"][:], in_=a_em[:])

                if q == SPB - 1:
                    osb = sb.tile([P, 136], f32, tag="osb")
                    nc.vector.tensor_copy(out=osb[:], in_=acc[:])
                    nc.sync.dma_start(out=outb[b * P:(b + 1) * P, :], in_=osb[:])

    nc.compile()
    return nc


def _host_prep(x, timestamps, src, dst, edge_type, edge_time, rel_table,
               Wq, bq, Wk, bk, Wv, bv, W1, b1, W2, b2, time_coeff,
               nmaxn=NMAXN):
    x = np.asarray(x, F32)
    timestamps = np.asarray(timestamps, F32)
    src = np.asarray(src).astype(np.int64)
    dst = np.asarray(dst).astype(np.int64)
    edge_type = np.asarray(edge_type).astype(np.int64)
    edge_time = np.asarray(edge_time, F32)
    Wq = np.asarray(Wq, F32); Wk = np.asarray(Wk, F32); Wv = np.asarray(Wv, F32)
    W1 = np.asarray(W1, F32); W2 = np.asarray(W2, F32)
    bq = np.asarray(bq, F32); b1 = np.asarray(b1, F32)
    bv = np.asarray(bv, F32); rel_table = np.asarray(rel_table, F32)

    invc = 1.0 / (abs(float(np.asarray(time_coeff))) + 1e-9)
    b2val = float(np.asarray(b2).reshape(-1)[0])
    # (d,h) column permutation for q/k/v spaces
    fprm = np.array([(f % NH) * HD + (f // NH) for f in range(H)])

    order = np.argsort(dst, kind="stable")
    dst_s = dst[order]
    src_s = src[order]
    et_s = edge_type[order]
    dm_s = -(timestamps[dst_s] - edge_time[order]) * invc
    counts = np.bincount(dst_s, minlength=N)
    cum = np.concatenate([[0], np.cumsum(counts)])

    nb = [0]
    for c in range(1, NCORES):
        nb.append(int(np.searchsorted(cum, E * c // NCORES)))
    nb.append(N)

    cores = []
    for c in range(NCORES):
        n0, n1 = nb[c], nb[c + 1]
        assert n1 - n0 <= nmaxn, (n0, n1)
        blocks = []
        n = n0
        while n < n1:
            bn = []
            edges = 0
            while n < n1 and len(bn) < P:
                cn = int(counts[n])
                if cn == 0:
                    n += 1
                    continue
                if edges + cn > EPB:
                    break
                bn.append(n)
                edges += cn
                n += 1
            if bn:
                blocks.append((bn, int(cum[bn[0]]), int(cum[bn[-1] + 1])))
        cores.append(blocks)
    nblk = max(len(bl) for bl in cores)

    def wrap16(flat, n):
        base = flat.reshape(n // 16, 16).T.astype(np.int16)
        return np.tile(base, (8, 1))

    cbm = np.zeros((P, NCB), F32)
    cbm[:, IOTA128R:IOTA128R + 128] = np.arange(P, dtype=F32)[None, :]
    cbm[:, WQ:WQ + 128] = Wq
    cbm[:, W1S:W1S + 128] = W1[:H]
    cbm[:, WKV:WKV + 128] = Wk
    cbm[:, WKV + 128:WKV + 256] = Wv[:, fprm]
    cbm[0:64, RELT:RELT + 128] = rel_table
    relp = rel_table @ W1[H:2 * H] + W1[2 * H]
    cbm[0:64, RELP:RELP + 128] = relp
    cbm[:, IDENT:IDENT + 128] = np.eye(P, dtype=F32)
    cbm[0, ONES64:ONES64 + 64] = 1.0
    cbm[:, ONES64 + 64] = W2[:, 0]
    cfm = np.zeros((P, NCF), F32)
    cfm[0:64, IOTA64C] = np.arange(64, dtype=F32)
    cfm[:, B1] = b1
    cfm[:, BQ025] = 0.25 * bq

    xg = np.ascontiguousarray(x.astype(BF))
    in_maps = []
    assembly = []
    for c in range(NCORES):
        n0 = nb[c]
        blocks = cores[c]
        ncn = nb[c + 1] - n0
        xtb = np.zeros((nmaxn, H), F32)
        xtb[:ncn] = x[n0:nb[c + 1]]
        xtm = np.ascontiguousarray(xtb.T).astype(BF)

        idx_a = np.zeros((P, nblk, P), np.int16)
        qix_a = np.zeros((P, nblk, P), np.int16)
        dl_a = np.full((P, nblk, CH), PAD_SLOT, F32)
        dm_a = np.zeros((P, nblk, CH), F32)
        etr_a = np.zeros((nblk, EPB), F32)
        asmb = []
        for b, (bn, e0, e1) in enumerate(blocks):
            ne = e1 - e0
            bn_arr = np.asarray(bn)
            sl = slice(e0, e1)
            bsrc = src_s[sl]
            # partition edges: A -> table x[0:XSPLIT], B -> x[XB0:]
            isA = bsrc < XSPLIT
            isB = bsrc >= XB0
            mustA = np.flatnonzero(~isB)          # src < XB0
            mustB = np.flatnonzero(~isA)          # src >= XSPLIT
            both = np.flatnonzero(isA & isB)
            assert len(mustA) <= KA and len(mustB) <= EPB - KA, (len(mustA), len(mustB))
            takeA = KA - len(mustA)
            grpA = np.concatenate([mustA, both[:takeA]])
            grpB = np.concatenate([both[takeA:], mustB])
            perm = np.concatenate([grpA, grpB]).astype(np.int64)
            nA = len(grpA)
            slotA = np.arange(len(grpA))
            slotB = KA + np.arange(len(grpB))
            slot = np.concatenate([slotA, slotB])

            buf_ia = np.zeros(KA, np.int64)
            buf_ib = np.zeros(EPB - KA, np.int64)
            buf_ia[:nA] = bsrc[grpA]
            buf_ib[:len(grpB)] = bsrc[grpB] - XB0
            buf_qi = np.zeros(EPB, np.int64)
            buf_dl = np.full(EPB, PAD_SLOT, F32)
            buf_dm = np.zeros(EPB, F32)
            buf_et = np.zeros(EPB, np.int64)
            eidx = np.arange(e0, e1)[perm]
            buf_qi[slot] = dst_s[eidx] - n0
            buf_dl[slot] = np.searchsorted(bn_arr, dst_s[eidx]).astype(F32)
            buf_dm[slot] = dm_s[eidx]
            buf_et[slot] = et_s[eidx]

            idx_a[:, b, 0:P // 2] = wrap16(buf_ia, KA)
            idx_a[:, b, P // 2:P] = wrap16(buf_ib, EPB - KA)
            qix_a[:, b, :] = wrap16(buf_qi, EPB)
            dl_a[:, b, :] = buf_dl.reshape(CH, P).T
            dm_a[:, b, :] = buf_dm.reshape(CH, P).T
            etr_a[b, :] = buf_et.astype(F32)
            asmb.append(bn_arr)
        assembly.append(asmb)
        in_maps.append({
            "xg": xg,
            "xt": xtm,
            "cb": cbm.astype(BF),
            "cf": cfm,
            "idx": np.ascontiguousarray(idx_a.reshape(P, nblk * P)),
            "qix": np.ascontiguousarray(qix_a.reshape(P, nblk * P)),
            "dlp": np.ascontiguousarray(dl_a.reshape(P, nblk * CH)),
            "dmp": np.ascontiguousarray(dm_a.reshape(P, nblk * CH)),
            "etr": etr_a.astype(BF),
        })
    return in_maps, nblk, b2val, bv, assembly


def _run(inputs, trace=False):
    from concourse.bass_utils import run_bass_kernel_spmd
    in_maps, nblk, b2val, bv, assembly = _host_prep(**inputs)
    nc = _build(nblk, b2val)
    res = run_bass_kernel_spmd(nc, in_maps, list(range(NCORES)), trace=trace)
    out = np.zeros((N, H), F32)
    fprm = np.array([(f % NH) * HD + (f // NH) for f in range(H)])
    inv = np.argsort(fprm)  # (d,h) -> original (h,d) columns
    for c in range(NCORES):
        ob = res.results[c]["outb"]
        for b, bn_arr in enumerate(assembly[c]):
            rows = ob[b * P:b * P + len(bn_arr)]
            esum = rows[:, 0:8]
            vsum = rows[:, 8:136]
            den = np.repeat(esum, HD, axis=0).reshape(len(bn_arr), NH, HD)
            # vsum cols are (d,h): vsum[:, d*8+h]
            vdh = vsum.reshape(-1, HD, NH)
            o = vdh / np.maximum(esum[:, None, :], 1e-30)   # [n, d, h]
            out[bn_arr] = o.transpose(0, 2, 1).reshape(-1, H) + bv[None, :]
    return out, res, nc


def kernel(**inputs):
    out, _res, _nc = _run(inputs)
    return out

